# revision 21
# baseline (speedup 1.0000x reference)
"""TRN2 Bass kernel for nn_CTCPerSpeakerExtractorConcatSoftmax.

Shards the (B=4, K=2) problem as one (batch, speaker) pair per NeuronCore
(8 cores). Each core runs the full per-stream pipeline:
  X = x_m @ proj_w.T + b
  Wmix from logits softmax (K=2 -> sigmoid of blank-posterior difference)
  KV = LN(X); K,V,Q projections (LN gamma/beta folded into weights host-side)
  banded attention (|i-j|<=24) computed in transposed-scores layout
  out-proj + residual, LN + FFN(gelu) + residual
Large matmuls run in float32r (TF32-like, full PE rate at N>=256); the
attention-probability and FFN matmuls run in bf16.

SBUF note: Tile pools reserve one slot per tag, so disjoint-lifetime
tensors are deliberately overlaid onto shared slot tags (slotA..slotF).
"""

import contextlib

import numpy as np

import concourse.bass as bass
import concourse.tile as tile
from concourse import bacc, mybir
from concourse.bass_utils import run_bass_kernel_spmd

F32 = mybir.dt.float32
F32R = mybir.dt.float32r
BF16 = mybir.dt.bfloat16
AF = mybir.ActivationFunctionType
ALU = mybir.AluOpType

B, T, DIN, D, K, V, H, BAND = 4, 1024, 512, 512, 2, 500, 8, 24
HD = D // H          # 64
DF = 4 * D           # 2048 ffn hidden
TCH = T // 128       # 8 t-chunks
ICH = D // 128       # 4 feature chunks
FCH = DF // 128      # 16 ffn feature chunks
NC = 8               # cores
SCALE = 1.0 / np.sqrt(HD)
NEG = -30000.0       # additive mask value (exp(SCALE*NEG) == 0 in fp32)
EPS = 1e-5


WIN = 384  # t-window width of a scores^T tile


def _win0(j):
    # t-window start of scores^T s-chunk j: [128j-128, 128j+256) clamped
    return min(max(128 * j - 128, 0), T - WIN)


def _mask_delta(j):
    # col offset into the master mask for s-chunk j
    return 256 - (128 * j - _win0(j))


def build_program(blank_id: int, flags: tuple):
    nc = bacc.Bacc("TRN2", target_bir_lowering=False, num_devices=NC)

    def inp(name, shape):
        return nc.declare_dram_parameter(name, list(shape), F32, isOutput=False)

    d = {}
    d["xm_d"] = inp("xm", (T, DIN))           # x_m[b]
    d["lg_d"] = inp("lg", (K, T, V))          # logits[:, b], own speaker first
    d["pwT_d"] = inp("pwT", (DIN, D))         # proj_w.T
    d["bproj_d"] = inp("bproj", (D,))
    d["wkT_d"] = inp("wkT", (D, D))           # (wk * g_kv).T
    d["wvT_d"] = inp("wvT", (D, D))
    d["wqT_d"] = inp("wqT", (D, D))           # (wq * g_q).T
    d["bk_d"] = inp("bk", (D,))               # bk + wk @ b_kv
    d["bv_d"] = inp("bv", (D,))
    d["bq_d"] = inp("bq", (D,))
    d["woT_d"] = inp("woT", (D, D))           # out_w.T
    d["outb_d"] = inp("outb", (D,))
    d["w1T_d"] = inp("w1T", (D, DF))          # (ffn_w1 * g_ffn).T
    d["b1_d"] = inp("b1", (DF,))
    d["w2T_d"] = inp("w2T", (DF, D))          # ffn_w2.T
    d["b2_d"] = inp("b2", (D,))
    d["tag_d"] = inp("tag", (D,))             # spk_tags[kk]
    d["mask_d"] = inp("maskm", (128, 640))    # master additive band mask
    d["ident_d"] = inp("ident", (128, 128))
    d["yout_d"] = nc.declare_dram_parameter("yout", [T, D], F32, isOutput=True)

    with tile.TileContext(nc) as tc:
        _build_tile(tc, d, blank_id, flags)
    nc.compile()
    return nc


def _build_tile(tc, d, blank_id, flags):
    nc = tc.nc
    add_bproj, add_bkq, add_bv, add_outb, add_b1, add_b2 = flags
    xm_d, lg_d, pwT_d, bproj_d = d["xm_d"], d["lg_d"], d["pwT_d"], d["bproj_d"]
    wkT_d, wvT_d, wqT_d = d["wkT_d"], d["wvT_d"], d["wqT_d"]
    bk_d, bv_d, bq_d = d["bk_d"], d["bv_d"], d["bq_d"]
    woT_d, outb_d, w1T_d, b1_d = d["woT_d"], d["outb_d"], d["w1T_d"], d["b1_d"]
    w2T_d, b2_d, tag_d = d["w2T_d"], d["b2_d"], d["tag_d"]
    mask_d, ident_d, yout_d = d["mask_d"], d["ident_d"], d["yout_d"]

    def copy_op(eng, out, in_):
        if eng is nc.scalar:
            nc.scalar.activation(out=out, in_=in_, func=AF.Identity)
        else:
            eng.tensor_copy(out=out, in_=in_)

    def add_col(eng, out, in0, col):
        if eng is nc.scalar:
            nc.scalar.activation(out=out, in_=in0, func=AF.Identity, bias=col)
        else:
            eng.tensor_scalar_add(out=out, in0=in0, scalar1=col)

    ctx = contextlib.ExitStack()
    with ctx:
        consts = ctx.enter_context(tc.tile_pool(name="consts", bufs=1))
        persist = ctx.enter_context(tc.tile_pool(name="persist", bufs=1))
        scratch = ctx.enter_context(tc.tile_pool(name="scratch", bufs=3))
        wload = ctx.enter_context(tc.tile_pool(name="wload", bufs=2))
        stats = ctx.enter_context(tc.tile_pool(name="stats", bufs=6))
        ptp = ctx.enter_context(tc.tile_pool(name="ptp", bufs=12))
        pp = ctx.enter_context(tc.tile_pool(name="pp", bufs=3, space="PSUM"))
        pps = ctx.enter_context(tc.tile_pool(name="pps", bufs=3, space="PSUM"))
        ppc = ctx.enter_context(tc.tile_pool(name="ppc", bufs=2, space="PSUM"))

        def bcast_row(dram, n, name):
            t = consts.tile([128, n], F32, tag=name)
            nc.gpsimd.dma_start(out=t, in_=dram.ap().partition_broadcast(128))
            return t

        def col_tile(dram, nch, name):
            # [C*128] dram vector -> [128, C] sbuf (col c = chunk c)
            t = consts.tile([128, nch], F32, tag=name)
            nc.sync.dma_start(out=t, in_=dram.rearrange("(c p) -> p c", p=128))
            return t

        # ---- constants ----
        ident_f = consts.tile([128, 128], F32, tag="ident_f")
        nc.sync.dma_start(out=ident_f, in_=ident_d[:, :])
        ident_r = consts.tile([128, 128], F32R, tag="ident_r")
        nc.vector.tensor_copy(out=ident_r, in_=ident_f)
        ident_b = consts.tile([128, 128], BF16, tag="ident_b")
        nc.vector.tensor_copy(out=ident_b, in_=ident_f)
        mask_f = consts.tile([128, 640], F32, tag="mask_f")
        nc.sync.dma_start(out=mask_f, in_=mask_d[:, :])
        mask_b = consts.tile([128, 640], BF16, tag="mask_b")
        # multiplicative 0/1 bf16 mask: relu(additive/NEG + 1) == band indicator
        nc.scalar.activation(out=mask_b, in_=mask_f, func=AF.Relu,
                             scale=-1.0 / NEG, bias=1.0)
        eps_t = consts.tile([128, 1], F32, tag="eps")
        nc.vector.memset(eps_t, EPS)
        bproj_rep = bcast_row(bproj_d, D, "bproj_rep") if add_bproj else None
        bv_rep = bcast_row(bv_d, D, "bv_rep") if add_bv else None
        tag_rep = bcast_row(tag_d, D, "tag_rep")
        outb_rep = bcast_row(outb_d, D, "outb_rep") if add_outb else None
        b2_rep = bcast_row(b2_d, D, "b2_rep") if add_b2 else None
        bk_c = col_tile(bk_d, ICH, "bk_c") if add_bkq else None
        bq_c = col_tile(bq_d, ICH, "bq_c") if add_bkq else None
        b1_c = col_tile(b1_d, FCH, "b1_c") if add_b1 else None

        # ---- weights ----
        def load_w_r(dram, nch, out_free, tag):
            # f32 HBM -> f32r SBUF [128, nch, out_free] via gpsimd cast
            w = persist.tile([128, nch, out_free], F32R, tag=tag)
            for c in range(nch):
                for p in range(out_free // 512):
                    s = wload.tile([128, 512], F32, tag="wl512")
                    nc.sync.dma_start(
                        out=s, in_=dram[128 * c:128 * (c + 1),
                                        512 * p:512 * (p + 1)])
                    nc.gpsimd.tensor_copy(out=w[:, c, 512 * p:512 * (p + 1)],
                                          in_=s)
            return w

        def load_w_b(dram, nch, out_free, tag):
            # f32 HBM -> bf16 SBUF directly (SWDGE dtype-cast DMA)
            w = persist.tile([128, nch, out_free], BF16, tag=tag)
            nc.gpsimd.dma_start(
                out=w, in_=dram.rearrange("(c p) o -> p c o", p=128))
            return w

        # ---- phase A: blank posteriors p_s and Wmix (exp table set) ----
        p_sb = consts.tile([128, K, TCH], F32, tag="p_sb")
        for s in range(K):
            for tch in range(TCH):
                lg_t = scratch.tile([128, V], F32, tag="lg")
                nc.sync.dma_start(out=lg_t,
                                  in_=lg_d[s, 128 * tch:128 * (tch + 1), :])
                ex = scratch.tile([128, V], BF16, tag="lgex")
                sm = stats.tile([128, 1], F32, tag="sumexp")
                nc.scalar.activation(out=ex, in_=lg_t, func=AF.Exp, accum_out=sm)
                rc = stats.tile([128, 1], F32, tag="rcs")
                nc.vector.reciprocal(out=rc, in_=sm)
                nc.vector.tensor_scalar_mul(
                    out=p_sb[:, s, tch:tch + 1],
                    in0=ex[:, blank_id:blank_id + 1], scalar1=rc)
        # wmix = sigmoid(p_other - p_own) = 1/(1+exp(p_own - p_other))
        wmix = consts.tile([128, TCH], F32, tag="wmix")
        for tch in range(TCH):
            df = stats.tile([128, 1], F32, tag="wdiff")
            nc.vector.tensor_sub(out=df, in0=p_sb[:, 0, tch:tch + 1],
                                 in1=p_sb[:, 1, tch:tch + 1])
            en = stats.tile([128, 1], F32, tag="wexp")
            nc.scalar.activation(out=en, in_=df, func=AF.Exp)
            ep1 = stats.tile([128, 1], F32, tag="wexp1")
            nc.vector.tensor_scalar_add(out=ep1, in0=en, scalar1=1.0)
            nc.vector.reciprocal(out=wmix[:, tch:tch + 1], in_=ep1)

        # ---- phase B: xm^T then X = xm @ proj_w.T + b ----
        xmT = persist.tile([128, ICH, T], F32R, tag="slotA")
        pwT = load_w_r(pwT_d, ICH, D, "w8_a")
        for tch in range(TCH):
            xm_t = scratch.tile([128, DIN], F32, tag="xm_t")
            nc.sync.dma_start(out=xm_t, in_=xm_d[128 * tch:128 * (tch + 1), :])
            ps = pp.tile([128, 512], F32, tag="psA")
            for ic in range(ICH):
                nc.tensor.transpose(ps[:, 128 * ic:128 * (ic + 1)],
                                    xm_t[:, 128 * ic:128 * (ic + 1)], ident_f)
            eng = nc.vector if tch % 2 == 0 else nc.scalar
            copy_op(eng, xmT[:, :, 128 * tch:128 * (tch + 1)],
                    ps.rearrange("p (c f) -> p c f", c=ICH))

        X = persist.tile([128, TCH, D], F32, tag="slotB")
        for tch in range(TCH):
            ps = pp.tile([128, D], F32, tag="psA")
            for ic in range(ICH):
                nc.tensor.matmul(ps, xmT[:, ic, 128 * tch:128 * (tch + 1)],
                                 pwT[:, ic, :], start=(ic == 0),
                                 stop=(ic == ICH - 1))
            if add_bproj:
                nc.vector.tensor_add(out=X[:, tch, :], in0=ps, in1=bproj_rep)
            else:
                copy_op(nc.scalar if tch % 2 else nc.vector, X[:, tch, :], ps)

        # ---- LN helper (t-major, gamma/beta folded into consumers) ----
        def ln_rstd(x_t, tagn):
            st6 = stats.tile([128, 6], F32, tag=f"st6_{tagn}")
            nc.vector.bn_stats(out=st6, in_=x_t)
            mv = stats.tile([128, 2], F32, tag=f"mv_{tagn}")
            nc.vector.bn_aggr(out=mv, in_=st6)
            sd = stats.tile([128, 1], F32, tag=f"sd_{tagn}")
            nc.scalar.activation(out=sd, in_=mv[:, 1:2], func=AF.Sqrt, bias=eps_t)
            rs = stats.tile([128, 1], F32, tag=f"rs_{tagn}")
            nc.vector.reciprocal(out=rs, in_=sd)
            return mv[:, 0:1], rs

        def transpose_into(dst, src_t, tch, idt, dt, eng):
            # src_t: [128, 512] t-major -> dst[:, :, 128*tch:+128] feature-major
            ps = pp.tile([128, 512], dt, tag="psA")
            for ic in range(ICH):
                nc.tensor.transpose(ps[:, 128 * ic:128 * (ic + 1)],
                                    src_t[:, 128 * ic:128 * (ic + 1)], idt)
            copy_op(eng, dst[:, :, 128 * tch:128 * (tch + 1)],
                    ps.rearrange("p (c f) -> p c f", c=ICH))

        # ---- phase C: KV = LN(X) -> KV^T ----
        KVT = persist.tile([128, ICH, T], F32R, tag="slotC")
        for tch in range(TCH):
            mean, rs = ln_rstd(X[:, tch, :], "kv")
            kv_t = scratch.tile([128, D], F32R, tag="norm_t")
            nc.vector.tensor_scalar(out=kv_t, in0=X[:, tch, :], scalar1=mean,
                                    scalar2=rs, op0=ALU.subtract, op1=ALU.mult)
            transpose_into(KVT, kv_t, tch, ident_r, F32R, nc.scalar)

        # ---- Xk = X * wmix + tag ----
        Xk = persist.tile([128, TCH, D], F32, tag="slotD")
        for tch in range(TCH):
            nc.vector.tensor_scalar_mul(out=Xk[:, tch, :], in0=X[:, tch, :],
                                        scalar1=wmix[:, tch:tch + 1])
            nc.gpsimd.tensor_add(out=Xk[:, tch, :], in0=Xk[:, tch, :],
                                  in1=tag_rep)

        # ---- LN_q(Xk) -> Qin^T ----
        QinT = persist.tile([128, ICH, T], F32R, tag="slotE")
        for tch in range(TCH):
            mean, rs = ln_rstd(Xk[:, tch, :], "q")
            qi_t = scratch.tile([128, D], F32R, tag="norm_t")
            nc.vector.tensor_scalar(out=qi_t, in0=Xk[:, tch, :], scalar1=mean,
                                    scalar2=rs, op0=ALU.subtract, op1=ALU.mult)
            transpose_into(QinT, qi_t, tch, ident_r, F32R, nc.scalar)

        # ---- projections: K^T, Q^T, V_aug ----
        wkT = load_w_r(wkT_d, ICH, D, "w8_b")
        wvT = load_w_r(wvT_d, ICH, D, "w8_c")
        wqT = load_w_r(wqT_d, ICH, D, "w8_d")
        KT = persist.tile([128, ICH, T], F32R, tag="slotA")
        QT = persist.tile([128, ICH, T], F32R, tag="slotB")
        for (dst, wT, bc, src) in ((KT, wkT, bk_c, KVT), (QT, wqT, bq_c, QinT)):
            for oc in range(ICH):
                for th in range(2):
                    ps = pp.tile([128, 512], F32, tag="psA")
                    for ic in range(ICH):
                        nc.tensor.matmul(
                            ps, wT[:, ic, 128 * oc:128 * (oc + 1)],
                            src[:, ic, 512 * th:512 * (th + 1)],
                            start=(ic == 0), stop=(ic == ICH - 1))
                    eng = nc.vector if (oc + th) % 2 == 0 else nc.scalar
                    if add_bkq:
                        add_col(eng, dst[:, oc, 512 * th:512 * (th + 1)], ps,
                                bc[:, oc:oc + 1])
                    else:
                        copy_op(eng, dst[:, oc, 512 * th:512 * (th + 1)], ps)

        VA = persist.tile([128, TCH, H, HD + 1], BF16, tag="va")
        for tch in range(TCH):
            ps = pp.tile([128, D], F32, tag="psA")
            for ic in range(ICH):
                nc.tensor.matmul(ps, KVT[:, ic, 128 * tch:128 * (tch + 1)],
                                 wvT[:, ic, :], start=(ic == 0),
                                 stop=(ic == ICH - 1))
            if add_bv:
                nc.vector.tensor_add(
                    out=VA[:, tch, :, 0:HD],
                    in0=ps.rearrange("p (h f) -> p h f", h=H),
                    in1=bv_rep.rearrange("p (h f) -> p h f", h=H))
            else:
                copy_op(nc.scalar if tch % 2 else nc.vector,
                        VA[:, tch, :, 0:HD],
                        ps.rearrange("p (h f) -> p h f", h=H))
        nc.gpsimd.memset(VA[:, :, :, HD:HD + 1], 1.0)

        # ---- attention (scores born transposed: [s, t]) ----
        # persistent PT buffer: out-of-band cols stay zero forever; each
        # (h, j) rewrites only its band slice [b0, b0+176).
        PT = persist.tile([128, TCH, WIN], BF16, tag="ptbuf")
        nc.vector.memset(PT, 0.0)

        def _band(j):
            # in-band col envelope of scores^T tile j: width 152/176
            off = 128 * j - _win0(j)
            b0 = max(off - 24, 0)
            b1 = min(off + 152, WIN)
            return b0, b1 - b0

        ctx_sb = persist.tile([128, TCH, D], F32R, tag="slotC")
        for h in range(H):
            hp, hc = (h % 2) * 64, h // 2
            for j in range(TCH):
                w0, dl = _win0(j), _mask_delta(j)
                b0, bw = _band(j)
                ps = pps.tile([128, WIN], F32, tag="ps_s")
                nc.tensor.matmul(ps, KT[hp:hp + 64, hc, 128 * j:128 * (j + 1)],
                                 QT[hp:hp + 64, hc, w0:w0 + WIN],
                                 start=True, stop=True)
                nc.scalar.activation(out=PT[:, j, b0:b0 + bw],
                                     in_=ps[:, b0:b0 + bw], func=AF.Exp,
                                     scale=SCALE)
                meng = nc.vector if j % 2 == 0 else nc.gpsimd
                meng.tensor_mul(out=PT[:, j, b0:b0 + bw],
                                in0=PT[:, j, b0:b0 + bw],
                                in1=mask_b[:, dl + b0:dl + b0 + bw])
            for tch in range(TCH):
                pc = ppc.tile([128, HD + 1], F32, tag="ps_c")
                js = [j for j in (tch - 1, tch, tch + 1) if 0 <= j < TCH]
                for n, j in enumerate(js):
                    cj = 128 * tch - _win0(j)
                    nc.tensor.matmul(pc, PT[:, j, cj:cj + 128],
                                     VA[:, j, h, :], start=(n == 0),
                                     stop=(n == len(js) - 1),
                                     skip_group_check=True)
                rc = stats.tile([128, 1], F32, tag="rc_ctx")
                nc.vector.reciprocal(out=rc, in_=pc[:, HD:HD + 1])
                nc.vector.tensor_scalar_mul(
                    out=ctx_sb[:, tch, 64 * h:64 * (h + 1)],
                    in0=pc[:, 0:HD], scalar1=rc)

        # ---- ctx^T, out-proj, residual ----
        woT = load_w_r(woT_d, ICH, D, "w8_a")
        ctxT = persist.tile([128, ICH, T], F32R, tag="slotE")
        for tch in range(TCH):
            transpose_into(ctxT, ctx_sb[:, tch, :], tch, ident_r, F32R,
                           nc.scalar)
        y1 = persist.tile([128, TCH, D], F32, tag="slotF")
        for tch in range(TCH):
            ps = pp.tile([128, D], F32, tag="psA")
            for ic in range(ICH):
                nc.tensor.matmul(ps, ctxT[:, ic, 128 * tch:128 * (tch + 1)],
                                 woT[:, ic, :], start=(ic == 0),
                                 stop=(ic == ICH - 1))
            nc.vector.tensor_add(out=y1[:, tch, :], in0=ps, in1=Xk[:, tch, :])
            if outb_rep is not None:
                nc.gpsimd.tensor_add(out=y1[:, tch, :], in0=y1[:, tch, :],
                                     in1=outb_rep)

        # ---- FFN (bf16 weights/hidden) ----
        w1T = load_w_b(w1T_d, ICH, DF, "slotA")
        w2T = load_w_b(w2T_d, FCH, D, "slotE")
        FT = persist.tile([128, ICH, T], BF16, tag="slotD")
        for tch in range(TCH):
            mean, rs = ln_rstd(y1[:, tch, :], "f")
            f_t = scratch.tile([128, D], BF16, tag="norm_t")
            nc.vector.tensor_scalar(out=f_t, in0=y1[:, tch, :], scalar1=mean,
                                    scalar2=rs, op0=ALU.subtract, op1=ALU.mult)
            transpose_into(FT, f_t, tch, ident_b, BF16, nc.scalar)
        for th in range(2):
            h1g = persist.tile([128, FCH, 512], BF16, tag="slotB")
            for oc in range(FCH):
                ps = pp.tile([128, 512], F32, tag="psA")
                for ic in range(ICH):
                    nc.tensor.matmul(ps, w1T[:, ic, 128 * oc:128 * (oc + 1)],
                                     FT[:, ic, 512 * th:512 * (th + 1)],
                                     start=(ic == 0), stop=(ic == ICH - 1))
                nc.scalar.activation(out=h1g[:, oc, :], in_=ps, func=AF.Gelu,
                                     bias=(b1_c[:, oc:oc + 1] if add_b1
                                           else 0.0))
            for tq in range(4):
                tch = 4 * th + tq
                ps = pp.tile([128, D], F32, tag="psA")
                for i in range(FCH):
                    nc.tensor.matmul(ps, h1g[:, i, 128 * tq:128 * (tq + 1)],
                                     w2T[:, i, :], start=(i == 0),
                                     stop=(i == FCH - 1))
                yo = scratch.tile([128, D], F32, tag="yo")
                nc.vector.tensor_add(out=yo, in0=ps, in1=y1[:, tch, :])
                if b2_rep is not None:
                    nc.gpsimd.tensor_add(out=yo, in0=yo, in1=b2_rep)
                nc.sync.dma_start(out=yout_d[128 * tch:128 * (tch + 1), :],
                                  in_=yo)

# ---------------- host side ----------------

_PROG_CACHE = {}


def _get_program(blank_id, flags):
    key = (blank_id, flags)
    if key not in _PROG_CACHE:
        _PROG_CACHE[key] = build_program(blank_id, flags)
    return _PROG_CACHE[key]


def _np(x):
    return np.asarray(x, dtype=np.float32)


def make_mask_master():
    # master additive band mask [128, 640]: 0 where 232 <= c - r <= 280
    r = np.arange(128)[:, None]
    c = np.arange(640)[None, :]
    onb = (c - r >= 232) & (c - r <= 280)
    return np.where(onb, 0.0, NEG).astype(np.float32)


def host_prepare(inputs):
    """Fold LN gamma/beta into weights; return shared input map + flags."""
    proj_w, proj_b = _np(inputs["proj_w"]), _np(inputs["proj_b"])
    lnq_g, lnq_b = _np(inputs["lnq_g"]), _np(inputs["lnq_b"])
    lnkv_g, lnkv_b = _np(inputs["lnkv_g"]), _np(inputs["lnkv_b"])
    in_w, in_b = _np(inputs["in_w"]), _np(inputs["in_b"])
    out_w, out_b = _np(inputs["out_w"]), _np(inputs["out_b"])
    ffn_ln_g, ffn_ln_b = _np(inputs["ffn_ln_g"]), _np(inputs["ffn_ln_b"])
    ffn_w1, ffn_b1 = _np(inputs["ffn_w1"]), _np(inputs["ffn_b1"])
    ffn_w2, ffn_b2 = _np(inputs["ffn_w2"]), _np(inputs["ffn_b2"])

    wq, wk, wv = in_w[:D], in_w[D:2 * D], in_w[2 * D:]
    bq, bk, bv = in_b[:D], in_b[D:2 * D], in_b[2 * D:]

    shared = {
        "pwT": np.ascontiguousarray(proj_w.T), "bproj": proj_b,
        "wkT": np.ascontiguousarray((wk * lnkv_g[None, :]).T),
        "bk": bk + wk @ lnkv_b,
        "wvT": np.ascontiguousarray((wv * lnkv_g[None, :]).T),
        "bv": bv + wv @ lnkv_b,
        "wqT": np.ascontiguousarray((wq * lnq_g[None, :]).T),
        "bq": bq + wq @ lnq_b,
        "woT": np.ascontiguousarray(out_w.T), "outb": out_b,
        "w1T": np.ascontiguousarray((ffn_w1 * ffn_ln_g[None, :]).T),
        "b1": ffn_b1 + ffn_w1 @ ffn_ln_b,
        "w2T": np.ascontiguousarray(ffn_w2.T), "b2": ffn_b2,
        "maskm": make_mask_master(),
        "ident": np.eye(128, dtype=np.float32),
    }
    nz = lambda a: bool(np.any(np.asarray(a) != 0))
    flags = (nz(proj_b), nz(shared["bk"]) or nz(shared["bq"]),
             nz(shared["bv"]), nz(out_b), nz(shared["b1"]), nz(ffn_b2))
    return shared, flags


def make_in_maps(inputs):
    x_m = _np(inputs["x_m"])
    logits = _np(inputs["logits"])
    spk_tags = _np(inputs["spk_tags"])
    shared, flags = host_prepare(inputs)
    in_maps = []
    for c in range(NC):
        b, kk = c // K, c % K
        m = dict(shared)
        m["xm"] = np.ascontiguousarray(x_m[b])
        m["lg"] = np.ascontiguousarray(logits[[kk, 1 - kk], b])
        m["tag"] = np.ascontiguousarray(spk_tags[kk])
        in_maps.append(m)
    return in_maps, flags


def kernel(**inputs):
    blank_id = int(np.asarray(inputs["blank_id"]))
    in_maps, flags = make_in_maps(inputs)
    nc = _get_program(blank_id, flags)
    res = run_bass_kernel_spmd(nc, in_maps, list(range(NC)))
    out = np.empty((B, K * T, D), dtype=np.float32)
    for c in range(NC):
        b, kk = c // K, c % K
        out[b, kk * T:(kk + 1) * T, :] = res.results[c]["yout"]
    return out


# revision 22
# speedup vs baseline: 97.2468x; 97.2468x over previous
"""TRN2 Bass kernel for nn_CTCPerSpeakerExtractorConcatSoftmax.

Shards the (B=4, K=2) problem as one (batch, speaker) pair per NeuronCore
(8 cores). Each core runs the full per-stream pipeline:
  X = x_m @ proj_w.T + b
  Wmix from logits softmax (K=2 -> sigmoid of blank-posterior difference)
  KV = LN(X); K,V,Q projections (LN gamma/beta folded into weights host-side)
  banded attention (|i-j|<=24) computed in transposed-scores layout
  out-proj + residual, LN + FFN(gelu) + residual
Large matmuls run in float32r (TF32-like, full PE rate at N>=256); the
attention-probability and FFN matmuls run in bf16.

SBUF note: Tile pools reserve one slot per tag, so disjoint-lifetime
tensors are deliberately overlaid onto shared slot tags (slotA..slotF).
"""

import contextlib

import numpy as np

import concourse.bass as bass
import concourse.tile as tile
from concourse import bacc, mybir
from concourse.bass_utils import run_bass_kernel_spmd

F32 = mybir.dt.float32
F32R = mybir.dt.float32r
BF16 = mybir.dt.bfloat16
AF = mybir.ActivationFunctionType
ALU = mybir.AluOpType

B, T, DIN, D, K, V, H, BAND = 4, 1024, 512, 512, 2, 500, 8, 24
HD = D // H          # 64
DF = 4 * D           # 2048 ffn hidden
TCH = T // 128       # 8 t-chunks
ICH = D // 128       # 4 feature chunks
FCH = DF // 128      # 16 ffn feature chunks
NC = 8               # cores
SCALE = 1.0 / np.sqrt(HD)
NEG = -30000.0       # additive mask value (exp(SCALE*NEG) == 0 in fp32)
EPS = 1e-5


WIN = 384  # t-window width of a scores^T tile


def _win0(j):
    # t-window start of scores^T s-chunk j: [128j-128, 128j+256) clamped
    return min(max(128 * j - 128, 0), T - WIN)


def _mask_delta(j):
    # col offset into the master mask for s-chunk j
    return 256 - (128 * j - _win0(j))


def build_program(blank_id: int, flags: tuple, repeat: int = 1):
    nc = bacc.Bacc("TRN2", target_bir_lowering=False, num_devices=NC)

    def inp(name, shape):
        return nc.declare_dram_parameter(name, list(shape), F32, isOutput=False)

    d = {}
    d["xm_d"] = inp("xm", (T, DIN))           # x_m[b]
    d["lg_d"] = inp("lg", (K, T, V))          # logits[:, b], own speaker first
    d["pwT_d"] = inp("pwT", (DIN, D))         # proj_w.T
    d["bproj_d"] = inp("bproj", (D,))
    d["wkT_d"] = inp("wkT", (D, D))           # (wk * g_kv).T
    d["wvT_d"] = inp("wvT", (D, D))
    d["wqT_d"] = inp("wqT", (D, D))           # (wq * g_q).T
    d["bk_d"] = inp("bk", (D,))               # bk + wk @ b_kv
    d["bv_d"] = inp("bv", (D,))
    d["bq_d"] = inp("bq", (D,))
    d["woT_d"] = inp("woT", (D, D))           # out_w.T
    d["outb_d"] = inp("outb", (D,))
    d["w1T_d"] = inp("w1T", (D, DF))          # (ffn_w1 * g_ffn).T
    d["b1_d"] = inp("b1", (DF,))
    d["w2T_d"] = inp("w2T", (DF, D))          # ffn_w2.T
    d["b2_d"] = inp("b2", (D,))
    d["tag_d"] = inp("tag", (D,))             # spk_tags[kk]
    d["mask_d"] = inp("maskm", (128, 640))    # master additive band mask
    d["ident_d"] = inp("ident", (128, 128))
    d["yout_d"] = nc.declare_dram_parameter("yout", [T, D], F32, isOutput=True)

    with tile.TileContext(nc) as tc:
        for _ in range(repeat):
            _build_tile(tc, d, blank_id, flags)
    nc.compile()
    return nc


def _build_tile(tc, d, blank_id, flags):
    nc = tc.nc
    add_bproj, add_bkq, add_bv, add_outb, add_b1, add_b2 = flags
    xm_d, lg_d, pwT_d, bproj_d = d["xm_d"], d["lg_d"], d["pwT_d"], d["bproj_d"]
    wkT_d, wvT_d, wqT_d = d["wkT_d"], d["wvT_d"], d["wqT_d"]
    bk_d, bv_d, bq_d = d["bk_d"], d["bv_d"], d["bq_d"]
    woT_d, outb_d, w1T_d, b1_d = d["woT_d"], d["outb_d"], d["w1T_d"], d["b1_d"]
    w2T_d, b2_d, tag_d = d["w2T_d"], d["b2_d"], d["tag_d"]
    mask_d, ident_d, yout_d = d["mask_d"], d["ident_d"], d["yout_d"]

    def copy_op(eng, out, in_):
        if eng is nc.scalar:
            nc.scalar.activation(out=out, in_=in_, func=AF.Identity)
        else:
            eng.tensor_copy(out=out, in_=in_)

    def add_col(eng, out, in0, col):
        if eng is nc.scalar:
            nc.scalar.activation(out=out, in_=in0, func=AF.Identity, bias=col)
        else:
            eng.tensor_scalar_add(out=out, in0=in0, scalar1=col)

    ctx = contextlib.ExitStack()
    with ctx:
        consts = ctx.enter_context(tc.tile_pool(name="consts", bufs=1))
        persist = ctx.enter_context(tc.tile_pool(name="persist", bufs=1))
        scratch = ctx.enter_context(tc.tile_pool(name="scratch", bufs=3))
        wload = ctx.enter_context(tc.tile_pool(name="wload", bufs=2))
        stats = ctx.enter_context(tc.tile_pool(name="stats", bufs=6))
        ptp = ctx.enter_context(tc.tile_pool(name="ptp", bufs=12))
        pp = ctx.enter_context(tc.tile_pool(name="pp", bufs=3, space="PSUM"))
        pps = ctx.enter_context(tc.tile_pool(name="pps", bufs=3, space="PSUM"))
        ppc = ctx.enter_context(tc.tile_pool(name="ppc", bufs=2, space="PSUM"))

        def bcast_row(dram, n, name):
            t = consts.tile([128, n], F32, tag=name)
            nc.gpsimd.dma_start(out=t, in_=dram.ap().partition_broadcast(128))
            return t

        def col_tile(dram, nch, name):
            # [C*128] dram vector -> [128, C] sbuf (col c = chunk c)
            t = consts.tile([128, nch], F32, tag=name)
            nc.sync.dma_start(out=t, in_=dram.rearrange("(c p) -> p c", p=128))
            return t

        # ---- constants ----
        ident_f = consts.tile([128, 128], F32, tag="ident_f")
        nc.sync.dma_start(out=ident_f, in_=ident_d[:, :])
        ident_r = consts.tile([128, 128], F32R, tag="ident_r")
        nc.vector.tensor_copy(out=ident_r, in_=ident_f)
        ident_b = consts.tile([128, 128], BF16, tag="ident_b")
        nc.vector.tensor_copy(out=ident_b, in_=ident_f)
        mask_f = consts.tile([128, 640], F32, tag="mask_f")
        nc.sync.dma_start(out=mask_f, in_=mask_d[:, :])
        mask_b = consts.tile([128, 640], BF16, tag="mask_b")
        # multiplicative 0/1 bf16 mask: relu(additive/NEG + 1) == band indicator
        nc.scalar.activation(out=mask_b, in_=mask_f, func=AF.Relu,
                             scale=-1.0 / NEG, bias=1.0)
        eps_t = consts.tile([128, 1], F32, tag="eps")
        nc.vector.memset(eps_t, EPS)
        bproj_rep = bcast_row(bproj_d, D, "bproj_rep") if add_bproj else None
        bv_rep = bcast_row(bv_d, D, "bv_rep") if add_bv else None
        tag_rep = bcast_row(tag_d, D, "tag_rep")
        outb_rep = bcast_row(outb_d, D, "outb_rep") if add_outb else None
        b2_rep = bcast_row(b2_d, D, "b2_rep") if add_b2 else None
        bk_c = col_tile(bk_d, ICH, "bk_c") if add_bkq else None
        bq_c = col_tile(bq_d, ICH, "bq_c") if add_bkq else None
        b1_c = col_tile(b1_d, FCH, "b1_c") if add_b1 else None

        # ---- weights ----
        def load_w_r(dram, nch, out_free, tag):
            # f32 HBM -> f32r SBUF [128, nch, out_free] via gpsimd cast
            w = persist.tile([128, nch, out_free], F32R, tag=tag)
            for c in range(nch):
                for p in range(out_free // 512):
                    s = wload.tile([128, 512], F32, tag="wl512")
                    nc.sync.dma_start(
                        out=s, in_=dram[128 * c:128 * (c + 1),
                                        512 * p:512 * (p + 1)])
                    nc.gpsimd.tensor_copy(out=w[:, c, 512 * p:512 * (p + 1)],
                                          in_=s)
            return w

        def load_w_b(dram, nch, out_free, tag):
            # f32 HBM -> bf16 SBUF directly (SWDGE dtype-cast DMA)
            w = persist.tile([128, nch, out_free], BF16, tag=tag)
            nc.gpsimd.dma_start(
                out=w, in_=dram.rearrange("(c p) o -> p c o", p=128))
            return w

        # ---- phase A: blank posteriors p_s and Wmix (exp table set) ----
        p_sb = consts.tile([128, K, TCH], F32, tag="p_sb")
        for s in range(K):
            for tch in range(TCH):
                lg_t = scratch.tile([128, V], F32, tag="lg")
                nc.sync.dma_start(out=lg_t,
                                  in_=lg_d[s, 128 * tch:128 * (tch + 1), :])
                ex = scratch.tile([128, V], BF16, tag="lgex")
                sm = stats.tile([128, 1], F32, tag="sumexp")
                nc.scalar.activation(out=ex, in_=lg_t, func=AF.Exp, accum_out=sm)
                rc = stats.tile([128, 1], F32, tag="rcs")
                nc.vector.reciprocal(out=rc, in_=sm)
                nc.vector.tensor_scalar_mul(
                    out=p_sb[:, s, tch:tch + 1],
                    in0=ex[:, blank_id:blank_id + 1], scalar1=rc)
        # wmix = sigmoid(p_other - p_own) = 1/(1+exp(p_own - p_other))
        wmix = consts.tile([128, TCH], F32, tag="wmix")
        for tch in range(TCH):
            df = stats.tile([128, 1], F32, tag="wdiff")
            nc.vector.tensor_sub(out=df, in0=p_sb[:, 0, tch:tch + 1],
                                 in1=p_sb[:, 1, tch:tch + 1])
            en = stats.tile([128, 1], F32, tag="wexp")
            nc.scalar.activation(out=en, in_=df, func=AF.Exp)
            ep1 = stats.tile([128, 1], F32, tag="wexp1")
            nc.vector.tensor_scalar_add(out=ep1, in0=en, scalar1=1.0)
            nc.vector.reciprocal(out=wmix[:, tch:tch + 1], in_=ep1)

        # ---- phase B: xm^T then X = xm @ proj_w.T + b ----
        xmT = persist.tile([128, ICH, T], F32R, tag="slotA")
        pwT = load_w_r(pwT_d, ICH, D, "w8_a")
        for tch in range(TCH):
            xm_t = scratch.tile([128, DIN], F32, tag="xm_t")
            nc.sync.dma_start(out=xm_t, in_=xm_d[128 * tch:128 * (tch + 1), :])
            ps = pp.tile([128, 512], F32, tag="psA")
            for ic in range(ICH):
                nc.tensor.transpose(ps[:, 128 * ic:128 * (ic + 1)],
                                    xm_t[:, 128 * ic:128 * (ic + 1)], ident_f)
            eng = nc.vector if tch % 2 == 0 else nc.scalar
            copy_op(eng, xmT[:, :, 128 * tch:128 * (tch + 1)],
                    ps.rearrange("p (c f) -> p c f", c=ICH))

        X = persist.tile([128, TCH, D], F32, tag="slotB")
        for tch in range(TCH):
            ps = pp.tile([128, D], F32, tag="psA")
            for ic in range(ICH):
                nc.tensor.matmul(ps, xmT[:, ic, 128 * tch:128 * (tch + 1)],
                                 pwT[:, ic, :], start=(ic == 0),
                                 stop=(ic == ICH - 1))
            if add_bproj:
                nc.vector.tensor_add(out=X[:, tch, :], in0=ps, in1=bproj_rep)
            else:
                copy_op(nc.scalar if tch % 2 else nc.vector, X[:, tch, :], ps)

        # ---- LN helper (t-major, gamma/beta folded into consumers) ----
        def ln_rstd(x_t, tagn):
            st6 = stats.tile([128, 6], F32, tag=f"st6_{tagn}")
            nc.vector.bn_stats(out=st6, in_=x_t)
            mv = stats.tile([128, 2], F32, tag=f"mv_{tagn}")
            nc.vector.bn_aggr(out=mv, in_=st6)
            sd = stats.tile([128, 1], F32, tag=f"sd_{tagn}")
            nc.scalar.activation(out=sd, in_=mv[:, 1:2], func=AF.Sqrt, bias=eps_t)
            rs = stats.tile([128, 1], F32, tag=f"rs_{tagn}")
            nc.vector.reciprocal(out=rs, in_=sd)
            return mv[:, 0:1], rs

        def transpose_into(dst, src_t, tch, idt, dt, eng):
            # src_t: [128, 512] t-major -> dst[:, :, 128*tch:+128] feature-major
            ps = pp.tile([128, 512], dt, tag="psA")
            for ic in range(ICH):
                nc.tensor.transpose(ps[:, 128 * ic:128 * (ic + 1)],
                                    src_t[:, 128 * ic:128 * (ic + 1)], idt)
            copy_op(eng, dst[:, :, 128 * tch:128 * (tch + 1)],
                    ps.rearrange("p (c f) -> p c f", c=ICH))

        # ---- phase C: KV = LN(X) -> KV^T ----
        KVT = persist.tile([128, ICH, T], F32R, tag="slotC")
        for tch in range(TCH):
            mean, rs = ln_rstd(X[:, tch, :], "kv")
            kv_t = scratch.tile([128, D], F32R, tag="norm_t")
            nc.vector.tensor_scalar(out=kv_t, in0=X[:, tch, :], scalar1=mean,
                                    scalar2=rs, op0=ALU.subtract, op1=ALU.mult)
            transpose_into(KVT, kv_t, tch, ident_r, F32R, nc.scalar)

        # ---- Xk = X * wmix + tag ----
        Xk = persist.tile([128, TCH, D], F32, tag="slotD")
        for tch in range(TCH):
            nc.vector.tensor_scalar_mul(out=Xk[:, tch, :], in0=X[:, tch, :],
                                        scalar1=wmix[:, tch:tch + 1])
            nc.gpsimd.tensor_add(out=Xk[:, tch, :], in0=Xk[:, tch, :],
                                  in1=tag_rep)

        # ---- LN_q(Xk) -> Qin^T ----
        QinT = persist.tile([128, ICH, T], F32R, tag="slotE")
        for tch in range(TCH):
            mean, rs = ln_rstd(Xk[:, tch, :], "q")
            qi_t = scratch.tile([128, D], F32R, tag="norm_t")
            nc.vector.tensor_scalar(out=qi_t, in0=Xk[:, tch, :], scalar1=mean,
                                    scalar2=rs, op0=ALU.subtract, op1=ALU.mult)
            transpose_into(QinT, qi_t, tch, ident_r, F32R, nc.scalar)

        # ---- projections: K^T, Q^T, V_aug ----
        wkT = load_w_r(wkT_d, ICH, D, "w8_b")
        wvT = load_w_r(wvT_d, ICH, D, "w8_c")
        wqT = load_w_r(wqT_d, ICH, D, "w8_d")
        KT = persist.tile([128, ICH, T], F32R, tag="slotA")
        QT = persist.tile([128, ICH, T], F32R, tag="slotB")
        for (dst, wT, bc, src) in ((KT, wkT, bk_c, KVT), (QT, wqT, bq_c, QinT)):
            for oc in range(ICH):
                for th in range(2):
                    ps = pp.tile([128, 512], F32, tag="psA")
                    for ic in range(ICH):
                        nc.tensor.matmul(
                            ps, wT[:, ic, 128 * oc:128 * (oc + 1)],
                            src[:, ic, 512 * th:512 * (th + 1)],
                            start=(ic == 0), stop=(ic == ICH - 1))
                    eng = nc.vector if (oc + th) % 2 == 0 else nc.scalar
                    if add_bkq:
                        add_col(eng, dst[:, oc, 512 * th:512 * (th + 1)], ps,
                                bc[:, oc:oc + 1])
                    else:
                        copy_op(eng, dst[:, oc, 512 * th:512 * (th + 1)], ps)

        VA = persist.tile([128, TCH, H, HD + 1], BF16, tag="va")
        for tch in range(TCH):
            ps = pp.tile([128, D], F32, tag="psA")
            for ic in range(ICH):
                nc.tensor.matmul(ps, KVT[:, ic, 128 * tch:128 * (tch + 1)],
                                 wvT[:, ic, :], start=(ic == 0),
                                 stop=(ic == ICH - 1))
            if add_bv:
                nc.vector.tensor_add(
                    out=VA[:, tch, :, 0:HD],
                    in0=ps.rearrange("p (h f) -> p h f", h=H),
                    in1=bv_rep.rearrange("p (h f) -> p h f", h=H))
            else:
                copy_op(nc.scalar if tch % 2 else nc.vector,
                        VA[:, tch, :, 0:HD],
                        ps.rearrange("p (h f) -> p h f", h=H))
        nc.gpsimd.memset(VA[:, :, :, HD:HD + 1], 1.0)

        # ---- attention (scores born transposed: [s, t]) ----
        # persistent PT buffer: out-of-band cols stay zero forever; each
        # (h, j) rewrites only its band slice [b0, b0+176).
        PT = persist.tile([128, TCH, WIN], BF16, tag="ptbuf")
        nc.vector.memset(PT, 0.0)

        def _band(j):
            # in-band col envelope of scores^T tile j: width 152/176
            off = 128 * j - _win0(j)
            b0 = max(off - 24, 0)
            b1 = min(off + 152, WIN)
            return b0, b1 - b0

        ctx_sb = persist.tile([128, TCH, D], F32R, tag="slotC")
        for h in range(H):
            hp, hc = (h % 2) * 64, h // 2
            for j in range(TCH):
                w0, dl = _win0(j), _mask_delta(j)
                b0, bw = _band(j)
                ps = pps.tile([128, WIN], F32, tag="ps_s")
                nc.tensor.matmul(ps, KT[hp:hp + 64, hc, 128 * j:128 * (j + 1)],
                                 QT[hp:hp + 64, hc, w0:w0 + WIN],
                                 start=True, stop=True)
                nc.scalar.activation(out=PT[:, j, b0:b0 + bw],
                                     in_=ps[:, b0:b0 + bw], func=AF.Exp,
                                     scale=SCALE)
                meng = nc.vector if j % 2 == 0 else nc.gpsimd
                meng.tensor_mul(out=PT[:, j, b0:b0 + bw],
                                in0=PT[:, j, b0:b0 + bw],
                                in1=mask_b[:, dl + b0:dl + b0 + bw])
            for tch in range(TCH):
                pc = ppc.tile([128, HD + 1], F32, tag="ps_c")
                js = [j for j in (tch - 1, tch, tch + 1) if 0 <= j < TCH]
                for n, j in enumerate(js):
                    cj = 128 * tch - _win0(j)
                    nc.tensor.matmul(pc, PT[:, j, cj:cj + 128],
                                     VA[:, j, h, :], start=(n == 0),
                                     stop=(n == len(js) - 1),
                                     skip_group_check=True)
                rc = stats.tile([128, 1], F32, tag="rc_ctx")
                nc.vector.reciprocal(out=rc, in_=pc[:, HD:HD + 1])
                nc.vector.tensor_scalar_mul(
                    out=ctx_sb[:, tch, 64 * h:64 * (h + 1)],
                    in0=pc[:, 0:HD], scalar1=rc)

        # ---- ctx^T, out-proj, residual ----
        woT = load_w_r(woT_d, ICH, D, "w8_a")
        ctxT = persist.tile([128, ICH, T], F32R, tag="slotE")
        for tch in range(TCH):
            transpose_into(ctxT, ctx_sb[:, tch, :], tch, ident_r, F32R,
                           nc.scalar)
        y1 = persist.tile([128, TCH, D], F32, tag="slotF")
        for tch in range(TCH):
            ps = pp.tile([128, D], F32, tag="psA")
            for ic in range(ICH):
                nc.tensor.matmul(ps, ctxT[:, ic, 128 * tch:128 * (tch + 1)],
                                 woT[:, ic, :], start=(ic == 0),
                                 stop=(ic == ICH - 1))
            nc.vector.tensor_add(out=y1[:, tch, :], in0=ps, in1=Xk[:, tch, :])
            if outb_rep is not None:
                nc.gpsimd.tensor_add(out=y1[:, tch, :], in0=y1[:, tch, :],
                                     in1=outb_rep)

        # ---- FFN (bf16 weights/hidden) ----
        w1T = load_w_b(w1T_d, ICH, DF, "slotA")
        w2T = load_w_b(w2T_d, FCH, D, "slotE")
        FT = persist.tile([128, ICH, T], BF16, tag="slotD")
        for tch in range(TCH):
            mean, rs = ln_rstd(y1[:, tch, :], "f")
            f_t = scratch.tile([128, D], BF16, tag="norm_t")
            nc.vector.tensor_scalar(out=f_t, in0=y1[:, tch, :], scalar1=mean,
                                    scalar2=rs, op0=ALU.subtract, op1=ALU.mult)
            transpose_into(FT, f_t, tch, ident_b, BF16, nc.scalar)
        for th in range(2):
            h1g = persist.tile([128, FCH, 512], BF16, tag="slotB")
            for oc in range(FCH):
                ps = pp.tile([128, 512], F32, tag="psA")
                for ic in range(ICH):
                    nc.tensor.matmul(ps, w1T[:, ic, 128 * oc:128 * (oc + 1)],
                                     FT[:, ic, 512 * th:512 * (th + 1)],
                                     start=(ic == 0), stop=(ic == ICH - 1))
                nc.scalar.activation(out=h1g[:, oc, :], in_=ps, func=AF.Gelu,
                                     bias=(b1_c[:, oc:oc + 1] if add_b1
                                           else 0.0))
            for tq in range(4):
                tch = 4 * th + tq
                ps = pp.tile([128, D], F32, tag="psA")
                for i in range(FCH):
                    nc.tensor.matmul(ps, h1g[:, i, 128 * tq:128 * (tq + 1)],
                                     w2T[:, i, :], start=(i == 0),
                                     stop=(i == FCH - 1))
                yo = scratch.tile([128, D], F32, tag="yo")
                nc.vector.tensor_add(out=yo, in0=ps, in1=y1[:, tch, :])
                if b2_rep is not None:
                    nc.gpsimd.tensor_add(out=yo, in0=yo, in1=b2_rep)
                nc.sync.dma_start(out=yout_d[128 * tch:128 * (tch + 1), :],
                                  in_=yo)

# ---------------- host side ----------------

_PROG_CACHE = {}


def _get_program(blank_id, flags):
    key = (blank_id, flags)
    if key not in _PROG_CACHE:
        _PROG_CACHE[key] = build_program(blank_id, flags)
    return _PROG_CACHE[key]


def _np(x):
    return np.asarray(x, dtype=np.float32)


def make_mask_master():
    # master additive band mask [128, 640]: 0 where 232 <= c - r <= 280
    r = np.arange(128)[:, None]
    c = np.arange(640)[None, :]
    onb = (c - r >= 232) & (c - r <= 280)
    return np.where(onb, 0.0, NEG).astype(np.float32)


def host_prepare(inputs):
    """Fold LN gamma/beta into weights; return shared input map + flags."""
    proj_w, proj_b = _np(inputs["proj_w"]), _np(inputs["proj_b"])
    lnq_g, lnq_b = _np(inputs["lnq_g"]), _np(inputs["lnq_b"])
    lnkv_g, lnkv_b = _np(inputs["lnkv_g"]), _np(inputs["lnkv_b"])
    in_w, in_b = _np(inputs["in_w"]), _np(inputs["in_b"])
    out_w, out_b = _np(inputs["out_w"]), _np(inputs["out_b"])
    ffn_ln_g, ffn_ln_b = _np(inputs["ffn_ln_g"]), _np(inputs["ffn_ln_b"])
    ffn_w1, ffn_b1 = _np(inputs["ffn_w1"]), _np(inputs["ffn_b1"])
    ffn_w2, ffn_b2 = _np(inputs["ffn_w2"]), _np(inputs["ffn_b2"])

    wq, wk, wv = in_w[:D], in_w[D:2 * D], in_w[2 * D:]
    bq, bk, bv = in_b[:D], in_b[D:2 * D], in_b[2 * D:]

    shared = {
        "pwT": np.ascontiguousarray(proj_w.T), "bproj": proj_b,
        "wkT": np.ascontiguousarray((wk * lnkv_g[None, :]).T),
        "bk": bk + wk @ lnkv_b,
        "wvT": np.ascontiguousarray((wv * lnkv_g[None, :]).T),
        "bv": bv + wv @ lnkv_b,
        "wqT": np.ascontiguousarray((wq * lnq_g[None, :]).T),
        "bq": bq + wq @ lnq_b,
        "woT": np.ascontiguousarray(out_w.T), "outb": out_b,
        "w1T": np.ascontiguousarray((ffn_w1 * ffn_ln_g[None, :]).T),
        "b1": ffn_b1 + ffn_w1 @ ffn_ln_b,
        "w2T": np.ascontiguousarray(ffn_w2.T), "b2": ffn_b2,
        "maskm": make_mask_master(),
        "ident": np.eye(128, dtype=np.float32),
    }
    nz = lambda a: bool(np.any(np.asarray(a) != 0))
    flags = (nz(proj_b), nz(shared["bk"]) or nz(shared["bq"]),
             nz(shared["bv"]), nz(out_b), nz(shared["b1"]), nz(ffn_b2))
    return shared, flags


def make_in_maps(inputs):
    x_m = _np(inputs["x_m"])
    logits = _np(inputs["logits"])
    spk_tags = _np(inputs["spk_tags"])
    shared, flags = host_prepare(inputs)
    in_maps = []
    for c in range(NC):
        b, kk = c // K, c % K
        m = dict(shared)
        m["xm"] = np.ascontiguousarray(x_m[b])
        m["lg"] = np.ascontiguousarray(logits[[kk, 1 - kk], b])
        m["tag"] = np.ascontiguousarray(spk_tags[kk])
        in_maps.append(m)
    return in_maps, flags


def kernel(**inputs):
    blank_id = int(np.asarray(inputs["blank_id"]))
    in_maps, flags = make_in_maps(inputs)
    nc = _get_program(blank_id, flags)
    res = run_bass_kernel_spmd(nc, in_maps, list(range(NC)))
    out = np.empty((B, K * T, D), dtype=np.float32)
    for c in range(NC):
        b, kk = c // K, c % K
        out[b, kk * T:(kk + 1) * T, :] = res.results[c]["yout"]
    return out
